# revision 6
# baseline (speedup 1.0000x reference)
"""Llama attention layer (B=2, S=2048, D=2048, H=16, DH=128) on 8 TRN2 NeuronCores.

Sharding: 2-way data parallel over batch x 4-way tensor parallel over heads.
Core c: batch g = c // 4, heads 4r..4r+3 where r = c % 4.
Projections are column-parallel (each core computes Q/K/V for its 4 heads),
attention is fully local per (batch, head), then the per-head attention
outputs (kept transposed, [dim, seq]) are AllGather'd within each 4-core
batch group, and o_proj is column-parallel: core c computes output columns
r*512..(r+1)*512 of its batch. Host concatenates - no host-side compute.

All matmul operands are bf16 (fp32 accumulation in PSUM); softmax runs
without max-subtraction (scores are O(6), exp is safe in fp32) with the
denominator accumulated exactly in PSUM via a ones-vector matmul.
"""

import os
import sys

for _p in ("/opt/trn_rl_repo", "/root/.axon_site/_ro/trn_rl_repo"):
    if os.path.isdir(_p) and _p not in sys.path:
        sys.path.append(_p)

import numpy as np
import ml_dtypes

import concourse.bass as bass
import concourse.tile as tile
import concourse.mybir as mybir
from concourse import bacc
from concourse.bass_utils import run_bass_kernel_spmd

F32 = mybir.dt.float32
BF16 = mybir.dt.bfloat16
AF = mybir.ActivationFunctionType

B, S, D, H, DH = 2, 2048, 2048, 16, 128
NCORES = 8
TP = 4                 # cores per batch group
HPC = H // TP          # heads per core = 4
SBLK = 512             # seq block (matmul moving size)
NSB = S // SBLK        # 4
DTILES = D // 128      # 16 contraction tiles
KT = S // 128          # 16 key tiles
OCOLS = D // TP        # 512 output columns per core
SCALE = 1.0 / float(np.sqrt(DH))

DT = BF16              # matmul operand dtype
NPDT = ml_dtypes.bfloat16


def _emit(tc):
    nc = tc.nc
    xT = nc.dram_tensor("xT", [D, S], DT, kind="ExternalInput").ap()
    wqT = nc.dram_tensor("wqT", [D, HPC * DH], DT, kind="ExternalInput").ap()
    wkT = nc.dram_tensor("wkT", [D, HPC * DH], DT, kind="ExternalInput").ap()
    wvT = nc.dram_tensor("wvT", [D, HPC * DH], DT, kind="ExternalInput").ap()
    woT = nc.dram_tensor("woT", [D, OCOLS], DT, kind="ExternalInput").ap()
    cosT = nc.dram_tensor("cosT", [DH, S], DT, kind="ExternalInput").ap()
    srotT = nc.dram_tensor("srotT", [DH, S], DT, kind="ExternalInput").ap()
    masks = nc.dram_tensor("masks", [4, 128, SBLK], DT, kind="ExternalInput").ap()
    outT = nc.dram_tensor("outT", [OCOLS, S], F32, kind="ExternalOutput").ap()

    vals_loc = nc.dram_tensor("vals_loc", [HPC * DH, S], DT).ap()
    vals_gath = nc.dram_tensor("vals_gath", [D, S], DT).ap()

    with tc.tile_pool(name="const", bufs=1) as cpool:
        cos_s = cpool.tile([128, S], DT, name="cos_s")
        nc.sync.dma_start(cos_s[:], cosT[:, :])
        srot_s = cpool.tile([128, S], DT, name="srot_s")
        nc.sync.dma_start(srot_s[:], srotT[:, :])
        mask_s = cpool.tile([128, 4, SBLK], DT, name="mask_s")
        nc.sync.dma_start(mask_s[:], masks.rearrange("m p s -> p m s"))
        # Full ones matrix: the denominator matmul ones.T @ st_exp yields the
        # softmax denominator replicated across all 128 partitions (DVE can't
        # broadcast along partitions, so produce it pre-broadcast).
        ones_s = cpool.tile([128, 128], DT, name="ones_s")
        nc.vector.memset(ones_s[:], 1.0)

        with tc.tile_pool(name="wqkv", bufs=1) as wpool, \
             tc.tile_pool(name="qkv", bufs=1) as qkvpool:
            wq_s = wpool.tile([128, DTILES, HPC * DH], DT, name="wq_s")
            nc.sync.dma_start(wq_s[:], wqT.rearrange("(t p) e -> p t e", p=128))
            wk_s = wpool.tile([128, DTILES, HPC * DH], DT, name="wk_s")
            nc.sync.dma_start(wk_s[:], wkT.rearrange("(t p) e -> p t e", p=128))
            wv_s = wpool.tile([128, DTILES, HPC * DH], DT, name="wv_s")
            nc.sync.dma_start(wv_s[:], wvT.rearrange("(t p) e -> p t e", p=128))

            qT = qkvpool.tile([128, HPC, S], DT, name="qT")
            kTt = qkvpool.tile([128, HPC, S], DT, name="kTt")
            v_s = qkvpool.tile([128, KT, HPC * DH], DT, name="v_s")

            # ---------------- projections + RoPE ----------------
            with tc.tile_pool(name="xs", bufs=2) as xpool, \
                 tc.tile_pool(name="pp", bufs=2, space="PSUM") as ppool, \
                 tc.tile_pool(name="rope", bufs=2) as rpool:
                for sb in range(NSB):
                    s0 = sb * SBLK
                    x_s = xpool.tile([128, DTILES, SBLK], DT, tag="x", name="x_s")
                    nc.sync.dma_start(
                        x_s[:], xT[:, s0:s0 + SBLK].rearrange("(t p) s -> p t s", p=128)
                    )
                    # Q / K sweep: 8 PSUM accumulators (2 tiles x 4 banks)
                    psq = ppool.tile([128, HPC, SBLK], F32, tag="ps", name="psq")
                    psk = ppool.tile([128, HPC, SBLK], F32, tag="ps", name="psk")
                    for dt_i in range(DTILES):
                        st_ = dt_i == 0
                        sp_ = dt_i == DTILES - 1
                        for h in range(HPC):
                            nc.tensor.matmul(
                                psq[:, h, :],
                                lhsT=wq_s[:, dt_i, h * DH:(h + 1) * DH],
                                rhs=x_s[:, dt_i, :],
                                start=st_, stop=sp_,
                            )
                        for h in range(HPC):
                            nc.tensor.matmul(
                                psk[:, h, :],
                                lhsT=wk_s[:, dt_i, h * DH:(h + 1) * DH],
                                rhs=x_s[:, dt_i, :],
                                start=st_, stop=sp_,
                            )

                    # RoPE: out = raw*cos + rot(raw)*srot  (partition dim = dh)
                    cos_b = cos_s[:, s0:s0 + SBLK].unsqueeze(1).broadcast_to(
                        [128, HPC, SBLK])
                    srot_b = srot_s[:, s0:s0 + SBLK].unsqueeze(1).broadcast_to(
                        [128, HPC, SBLK])
                    for ps, dst in ((psq, qT), (psk, kTt)):
                        raw = rpool.tile([128, HPC, SBLK], DT, tag="raw", name="raw")
                        nc.scalar.copy(raw[:], ps[:])
                        # rotate-half along partitions: engines can't shift
                        # partitions, DMA can.
                        rot = rpool.tile([128, HPC, SBLK], DT, tag="rot", name="rot")
                        nc.sync.dma_start(rot[0:64], raw[64:128])
                        nc.sync.dma_start(rot[64:128], raw[0:64])
                        rs = rpool.tile([128, HPC, SBLK], DT, tag="rs", name="rs")
                        nc.vector.tensor_mul(rs[:], rot[:], srot_b)
                        qc = rpool.tile([128, HPC, SBLK], DT, tag="qc", name="qc")
                        nc.vector.tensor_mul(qc[:], raw[:], cos_b)
                        nc.vector.tensor_add(dst[:, :, s0:s0 + SBLK], qc[:], rs[:])

                    # V sweep: x^T tiles stationary, W_v moving
                    psv = ppool.tile([128, 4, SBLK], F32, tag="ps", name="psv")
                    for dt_i in range(DTILES):
                        st_ = dt_i == 0
                        sp_ = dt_i == DTILES - 1
                        for st in range(4):  # seq sub-tiles of this block
                            nc.tensor.matmul(
                                psv[:, st, :],
                                lhsT=x_s[:, dt_i, st * 128:(st + 1) * 128],
                                rhs=wv_s[:, dt_i, :],
                                start=st_, stop=sp_,
                            )
                    for st in range(4):
                        nc.scalar.copy(v_s[:, sb * 4 + st, :], psv[:, st, :])

            # ---------------- attention ----------------
            with tc.tile_pool(name="aps", bufs=2, space="PSUM") as apsum, \
                 tc.tile_pool(name="att", bufs=3) as apool:
                for h in range(HPC):
                    for sqb in range(NSB):
                        sq0 = sqb * SBLK
                        nkt = 4 * (sqb + 1)
                        ps_av = apsum.tile([128, SBLK], F32, tag="av", name="ps_av")
                        ps_den = apsum.tile([128, SBLK], F32, tag="den",
                                            name="ps_den")
                        for kt in range(nkt):
                            ps_st = apsum.tile([128, SBLK], F32, tag="st",
                                               name="ps_st", bufs=3)
                            nc.tensor.matmul(
                                ps_st[:],
                                lhsT=kTt[:, h, kt * 128:(kt + 1) * 128],
                                rhs=qT[:, h, sq0:sq0 + SBLK],
                                start=True, stop=True,
                            )
                            st_e = apool.tile([128, SBLK], DT, tag="ste", name="st_e")
                            nc.scalar.activation(st_e[:], ps_st[:], AF.Exp,
                                                 scale=SCALE)
                            p = kt - (nkt - 4)
                            if p >= 0:  # diagonal 512-block: causal 0/1 mask
                                nc.vector.tensor_mul(st_e[:], st_e[:],
                                                     mask_s[:, p, :])
                            nc.tensor.matmul(
                                ps_av[:],
                                lhsT=v_s[:, kt, h * DH:(h + 1) * DH],
                                rhs=st_e[:],
                                start=(kt == 0), stop=(kt == nkt - 1),
                            )
                            nc.tensor.matmul(
                                ps_den[:],
                                lhsT=ones_s[:],
                                rhs=st_e[:],
                                start=(kt == 0), stop=(kt == nkt - 1),
                            )
                        rden = apool.tile([128, SBLK], F32, tag="rden", name="rden")
                        nc.vector.reciprocal(rden[:], ps_den[:])
                        vout = apool.tile([128, SBLK], DT, tag="vout", name="vout")
                        nc.vector.tensor_mul(vout[:], ps_av[:], rden[:])
                        nc.sync.dma_start(
                            vals_loc[h * DH:(h + 1) * DH, sq0:sq0 + SBLK], vout[:])

        # ---------------- all-gather (within each batch group) ----------------
        nc.gpsimd.collective_compute(
            "AllGather",
            mybir.AluOpType.bypass,
            replica_groups=[[0, 1, 2, 3], [4, 5, 6, 7]],
            ins=[vals_loc[:, :].opt()],
            outs=[vals_gath[:, :].opt()],
        )

        # ---------------- o_proj (column parallel) ----------------
        with tc.tile_pool(name="og", bufs=1) as ogpool, \
             tc.tile_pool(name="ops", bufs=2, space="PSUM") as opsum, \
             tc.tile_pool(name="ob", bufs=3) as obpool:
            wo_s = ogpool.tile([128, DTILES, OCOLS], DT, name="wo_s")
            nc.sync.dma_start(wo_s[:], woT.rearrange("(t p) e -> p t e", p=128))
            vg = ogpool.tile([128, DTILES, S], DT, name="vg")
            nc.sync.dma_start(vg[:], vals_gath.rearrange("(t p) s -> p t s", p=128))
            for ct in range(OCOLS // 128):
                ps_o = [
                    opsum.tile([128, SBLK], F32, tag=f"o{sb}", name="ps_o")
                    for sb in range(NSB)
                ]
                for dt_i in range(DTILES):
                    for sb in range(NSB):
                        nc.tensor.matmul(
                            ps_o[sb][:],
                            lhsT=wo_s[:, dt_i, ct * 128:(ct + 1) * 128],
                            rhs=vg[:, dt_i, sb * SBLK:(sb + 1) * SBLK],
                            start=(dt_i == 0), stop=(dt_i == DTILES - 1),
                        )
                for sb in range(NSB):
                    ob = obpool.tile([128, SBLK], F32, tag="ob", name="ob")
                    nc.scalar.copy(ob[:], ps_o[sb][:])
                    nc.sync.dma_start(
                        outT[ct * 128:(ct + 1) * 128, sb * SBLK:(sb + 1) * SBLK],
                        ob[:])


_NC_CACHE = None


def build_program():
    global _NC_CACHE
    if _NC_CACHE is not None:
        return _NC_CACHE
    nc = bacc.Bacc("TRN2", target_bir_lowering=False, debug=False,
                   enable_asserts=False, num_devices=NCORES)
    with tile.TileContext(nc) as tc:
        _emit(tc)
    nc.compile()
    _NC_CACHE = nc
    return nc


def _prep_inputs(x, cos, sin, Wq, Wk, Wv, Wo):
    """Build the 8 per-core input maps (host-side sharding only)."""
    x = np.asarray(x, dtype=np.float32)
    cos = np.asarray(cos, dtype=np.float32)
    sin = np.asarray(sin, dtype=np.float32)
    Wq = np.asarray(Wq, dtype=np.float32)
    Wk = np.asarray(Wk, dtype=np.float32)
    Wv = np.asarray(Wv, dtype=np.float32)
    Wo = np.asarray(Wo, dtype=np.float32)

    cosT = np.ascontiguousarray(cos.T).astype(NPDT)             # [128, S]
    sinT = np.ascontiguousarray(sin.T)
    srotT = np.concatenate([-sinT[:64], sinT[64:]], axis=0).astype(NPDT)

    iota = np.arange(SBLK)[None, :]
    rows = np.arange(128)[:, None]
    masks = np.stack(
        [(128 * p + rows <= iota) for p in range(4)]).astype(NPDT)  # [4,128,512]

    xTg = [np.ascontiguousarray(x[g].T).astype(NPDT) for g in range(B)]

    in_maps = []
    for c in range(NCORES):
        g, r = c // TP, c % TP
        hs = slice(r * HPC * DH, (r + 1) * HPC * DH)
        in_maps.append({
            "xT": xTg[g],
            "wqT": np.ascontiguousarray(Wq[hs].T).astype(NPDT),
            "wkT": np.ascontiguousarray(Wk[hs].T).astype(NPDT),
            "wvT": np.ascontiguousarray(Wv[hs].T).astype(NPDT),
            "woT": np.ascontiguousarray(Wo[r * OCOLS:(r + 1) * OCOLS].T).astype(NPDT),
            "cosT": cosT,
            "srotT": srotT,
            "masks": masks,
        })
    return in_maps


def run(inputs, trace=False, trace_cores=None):
    nc = build_program()
    in_maps = _prep_inputs(**inputs)
    res = run_bass_kernel_spmd(
        nc, in_maps, core_ids=list(range(NCORES)),
        trace=trace, trace_cores=trace_cores,
    )
    out = np.empty((B, S, D), dtype=np.float32)
    for c in range(NCORES):
        g, r = c // TP, c % TP
        out[g, :, r * OCOLS:(r + 1) * OCOLS] = res.results[c]["outT"].T
    return out, res


def kernel(**inputs):
    out, _ = run(inputs)
    return out


# revision 10
# speedup vs baseline: 1.1635x; 1.1635x over previous
"""Llama attention layer (B=2, S=2048, D=2048, H=16, DH=128) on 8 TRN2 NeuronCores.

Sharding: 2-way data parallel over batch x 4-way tensor parallel over heads.
Core c: batch g = c // 4, heads 4r..4r+3 where r = c % 4.
Projections are column-parallel (each core computes Q/K/V for its 4 heads),
attention is fully local per (batch, head), then the per-head attention
outputs (kept transposed, [dim, seq]) are AllGather'd within each 4-core
batch group in 4 seq-chunks (pipelined against attention compute), and
o_proj is column-parallel: core c computes output columns r*512..(r+1)*512
of its batch. Host concatenates - no host-side compute.

All matmul operands are bf16 (fp32 accumulation in PSUM); softmax runs
without max-subtraction (scores are O(6), exp is safe in fp32); the
denominator is accumulated on the vector engine in fp32 and reduced over
partitions with a single ones-matmul (f32r) per (head, seq-block).
"""

import os
import sys

for _p in ("/opt/trn_rl_repo", "/root/.axon_site/_ro/trn_rl_repo"):
    if os.path.isdir(_p) and _p not in sys.path:
        sys.path.append(_p)

import numpy as np
import ml_dtypes

import concourse.bass as bass
import concourse.tile as tile
import concourse.mybir as mybir
from concourse import bacc
from concourse.bass_utils import run_bass_kernel_spmd

F32 = mybir.dt.float32
F32R = mybir.dt.float32r
BF16 = mybir.dt.bfloat16
AF = mybir.ActivationFunctionType

B, S, D, H, DH = 2, 2048, 2048, 16, 128
NCORES = 8
TP = 4                 # cores per batch group
HPC = H // TP          # heads per core = 4
SBLK = 512             # seq block (matmul moving size)
NSB = S // SBLK        # 4
DTILES = D // 128      # 16 contraction tiles
KT = S // 128          # 16 key tiles
OCOLS = D // TP        # 512 output columns per core
SCALE = 1.0 / float(np.sqrt(DH))
DMA_SPLIT = 4          # split big input DMAs so compute starts early

DT = BF16              # matmul operand dtype
NPDT = ml_dtypes.bfloat16


def _split_load(nc, dst, src_2d, inner):
    """DMA a [D, inner] DRAM tensor into dst [128, DTILES, inner] in
    DMA_SPLIT chunks along the d-tile axis (lets consumers of early
    d-tiles start before the whole tensor has landed)."""
    step = DTILES // DMA_SPLIT
    for i in range(DMA_SPLIT):
        t0 = i * step
        nc.sync.dma_start(
            dst[:, t0:t0 + step, :],
            src_2d[t0 * 128:(t0 + step) * 128, :].rearrange(
                "(t p) s -> p t s", p=128),
        )


def _emit(tc):
    nc = tc.nc
    xT = nc.dram_tensor("xT", [D, S], DT, kind="ExternalInput").ap()
    wqT = nc.dram_tensor("wqT", [D, HPC * DH], DT, kind="ExternalInput").ap()
    wkT = nc.dram_tensor("wkT", [D, HPC * DH], DT, kind="ExternalInput").ap()
    wvT = nc.dram_tensor("wvT", [D, HPC * DH], DT, kind="ExternalInput").ap()
    woT = nc.dram_tensor("woT", [D, OCOLS], DT, kind="ExternalInput").ap()
    cosT = nc.dram_tensor("cosT", [DH, S], DT, kind="ExternalInput").ap()
    srotT = nc.dram_tensor("srotT", [DH, S], DT, kind="ExternalInput").ap()
    masks = nc.dram_tensor("masks", [4, 128, SBLK], DT, kind="ExternalInput").ap()
    outT = nc.dram_tensor("outT", [OCOLS, S], F32, kind="ExternalOutput").ap()

    # Per-seq-chunk bounce buffers for the pipelined AllGather.
    vloc = [nc.dram_tensor(f"vals_loc_{c}", [HPC * DH, SBLK], DT).ap()
            for c in range(NSB)]
    vgath = [nc.dram_tensor(f"vals_gath_{c}", [D, SBLK], DT).ap()
             for c in range(NSB)]

    with tc.tile_pool(name="const", bufs=1) as cpool:
        cos_s = cpool.tile([128, S], DT, name="cos_s")
        nc.sync.dma_start(cos_s[:], cosT[:, :])
        srot_s = cpool.tile([128, S], DT, name="srot_s")
        nc.sync.dma_start(srot_s[:], srotT[:, :])
        mask_s = cpool.tile([128, 4, SBLK], DT, name="mask_s")
        nc.sync.dma_start(mask_s[:], masks.rearrange("m p s -> p m s"))
        # f32 ones (used as f32r): one matmul per (head, seq-block) reduces
        # the DVE-accumulated softmax denominator over partitions, yielding
        # it pre-broadcast across all 128 partitions.
        ones_f = cpool.tile([128, 128], F32, name="ones_f")
        nc.vector.memset(ones_f[:], 1.0)
        ones_s = cpool.tile([128, 128], F32R, name="ones_s")
        nc.vector.tensor_copy(ones_s[:], ones_f[:])

        with tc.tile_pool(name="qkv", bufs=1) as qkvpool:
            qT = qkvpool.tile([128, HPC, S], DT, name="qT")
            kTt = qkvpool.tile([128, HPC, S], DT, name="kTt")
            v_s = qkvpool.tile([128, KT, HPC * DH], DT, name="v_s")

            # ---------------- projections + RoPE ----------------
            with tc.tile_pool(name="wqkv", bufs=1) as wpool, \
                 tc.tile_pool(name="xs", bufs=2) as xpool, \
                 tc.tile_pool(name="pp", bufs=2, space="PSUM") as ppool, \
                 tc.tile_pool(name="rope", bufs=2) as rpool:
                wq_s = wpool.tile([128, DTILES, HPC * DH], DT, name="wq_s")
                _split_load(nc, wq_s, wqT, HPC * DH)
                wk_s = wpool.tile([128, DTILES, HPC * DH], DT, name="wk_s")
                _split_load(nc, wk_s, wkT, HPC * DH)
                wv_s = wpool.tile([128, DTILES, HPC * DH], DT, name="wv_s")
                _split_load(nc, wv_s, wvT, HPC * DH)

                for sb in range(NSB):
                    s0 = sb * SBLK
                    x_s = xpool.tile([128, DTILES, SBLK], DT, tag="x", name="x_s")
                    _split_load(nc, x_s, xT[:, s0:s0 + SBLK], SBLK)
                    # Q / K sweep: 8 PSUM accumulators (2 tiles x 4 banks)
                    psq = ppool.tile([128, HPC, SBLK], F32, tag="ps", name="psq")
                    psk = ppool.tile([128, HPC, SBLK], F32, tag="ps", name="psk")
                    for dt_i in range(DTILES):
                        st_ = dt_i == 0
                        sp_ = dt_i == DTILES - 1
                        for h in range(HPC):
                            nc.tensor.matmul(
                                psq[:, h, :],
                                lhsT=wq_s[:, dt_i, h * DH:(h + 1) * DH],
                                rhs=x_s[:, dt_i, :],
                                start=st_, stop=sp_,
                            )
                        for h in range(HPC):
                            nc.tensor.matmul(
                                psk[:, h, :],
                                lhsT=wk_s[:, dt_i, h * DH:(h + 1) * DH],
                                rhs=x_s[:, dt_i, :],
                                start=st_, stop=sp_,
                            )

                    # RoPE: out = raw*cos + rot(raw)*srot  (partition dim = dh)
                    cos_b = cos_s[:, s0:s0 + SBLK].unsqueeze(1).broadcast_to(
                        [128, HPC, SBLK])
                    srot_b = srot_s[:, s0:s0 + SBLK].unsqueeze(1).broadcast_to(
                        [128, HPC, SBLK])
                    for ps, dst in ((psq, qT), (psk, kTt)):
                        raw = rpool.tile([128, HPC, SBLK], DT, tag="raw", name="raw")
                        nc.scalar.copy(raw[:], ps[:])
                        # rotate-half along partitions: engines can't shift
                        # partitions, DMA can.
                        rot = rpool.tile([128, HPC, SBLK], DT, tag="rot", name="rot")
                        nc.sync.dma_start(rot[0:64], raw[64:128])
                        nc.sync.dma_start(rot[64:128], raw[0:64])
                        rs = rpool.tile([128, HPC, SBLK], DT, tag="rs", name="rs")
                        nc.vector.tensor_mul(rs[:], rot[:], srot_b)
                        qc = rpool.tile([128, HPC, SBLK], DT, tag="qc", name="qc")
                        nc.vector.tensor_mul(qc[:], raw[:], cos_b)
                        nc.vector.tensor_add(dst[:, :, s0:s0 + SBLK], qc[:], rs[:])

                    # V sweep: x^T tiles stationary, W_v moving
                    psv = ppool.tile([128, 4, SBLK], F32, tag="ps", name="psv")
                    for dt_i in range(DTILES):
                        st_ = dt_i == 0
                        sp_ = dt_i == DTILES - 1
                        for st in range(4):  # seq sub-tiles of this block
                            nc.tensor.matmul(
                                psv[:, st, :],
                                lhsT=x_s[:, dt_i, st * 128:(st + 1) * 128],
                                rhs=wv_s[:, dt_i, :],
                                start=st_, stop=sp_,
                            )
                    for st in range(4):
                        nc.scalar.copy(v_s[:, sb * 4 + st, :], psv[:, st, :])

            # ---------------- attention + AG + o_proj, chunk-pipelined ------
            with tc.tile_pool(name="aps", bufs=2, space="PSUM") as apsum, \
                 tc.tile_pool(name="att", bufs=3) as apool, \
                 tc.tile_pool(name="og", bufs=1) as ogpool, \
                 tc.tile_pool(name="ops", bufs=2, space="PSUM") as opsum, \
                 tc.tile_pool(name="ob", bufs=3) as obpool:

                wo_s = ogpool.tile([128, DTILES, OCOLS], DT, name="wo_s")
                _split_load(nc, wo_s, woT, OCOLS)

                def attn_block(sqb):
                    sq0 = sqb * SBLK
                    nkt = 4 * (sqb + 1)
                    for h in range(HPC):
                        ps_av = apsum.tile([128, SBLK], F32, tag="av",
                                           name="ps_av")
                        dacc = apool.tile([128, SBLK], F32R, tag="dacc",
                                          name="dacc", bufs=2)
                        for kt in range(nkt):
                            ps_st = apsum.tile([128, SBLK], F32, tag="st",
                                               name="ps_st", bufs=3)
                            nc.tensor.matmul(
                                ps_st[:],
                                lhsT=kTt[:, h, kt * 128:(kt + 1) * 128],
                                rhs=qT[:, h, sq0:sq0 + SBLK],
                                start=True, stop=True,
                            )
                            st_e = apool.tile([128, SBLK], DT, tag="ste",
                                              name="st_e")
                            nc.scalar.activation(st_e[:], ps_st[:], AF.Exp,
                                                 scale=SCALE)
                            p = kt - (nkt - 4)
                            if p >= 0:  # diagonal 512-block: causal 0/1 mask
                                nc.vector.tensor_mul(st_e[:], st_e[:],
                                                     mask_s[:, p, :])
                            nc.tensor.matmul(
                                ps_av[:],
                                lhsT=v_s[:, kt, h * DH:(h + 1) * DH],
                                rhs=st_e[:],
                                start=(kt == 0), stop=(kt == nkt - 1),
                            )
                            if kt == 0:
                                nc.vector.tensor_copy(dacc[:], st_e[:])
                            else:
                                nc.vector.tensor_add(dacc[:], dacc[:], st_e[:])
                        # partition-reduce the denominator (result replicated
                        # across partitions), then normalize on the copy out.
                        ps_den = apsum.tile([128, SBLK], F32, tag="den",
                                            name="ps_den", bufs=1)
                        nc.tensor.matmul(
                            ps_den[:],
                            lhsT=ones_s[:],
                            rhs=dacc[:],
                            start=True, stop=True,
                        )
                        rden = apool.tile([128, SBLK], F32, tag="rden",
                                          name="rden", bufs=2)
                        nc.vector.reciprocal_approx_fast(rden[:], ps_den[:])
                        vout = apool.tile([128, SBLK], DT, tag="vout",
                                          name="vout")
                        nc.vector.tensor_mul(vout[:], ps_av[:], rden[:])
                        nc.sync.dma_start(
                            vloc[sqb][h * DH:(h + 1) * DH, :], vout[:])

                def ag_block(c):
                    nc.gpsimd.collective_compute(
                        "AllGather",
                        mybir.AluOpType.bypass,
                        replica_groups=[[0, 1, 2, 3], [4, 5, 6, 7]],
                        ins=[vloc[c][:, :].opt()],
                        outs=[vgath[c][:, :].opt()],
                    )

                def oproj_block(c):
                    vg = ogpool.tile([128, DTILES, SBLK], DT, tag="vg",
                                     name="vg", bufs=2)
                    _split_load(nc, vg, vgath[c], SBLK)
                    for ct in range(OCOLS // 128):
                        ps_o = opsum.tile([128, SBLK], F32, tag="o", name="ps_o")
                        for dt_i in range(DTILES):
                            nc.tensor.matmul(
                                ps_o[:],
                                lhsT=wo_s[:, dt_i, ct * 128:(ct + 1) * 128],
                                rhs=vg[:, dt_i, :],
                                start=(dt_i == 0), stop=(dt_i == DTILES - 1),
                            )
                        ob = obpool.tile([128, SBLK], F32, tag="ob", name="ob")
                        nc.scalar.copy(ob[:], ps_o[:])
                        nc.sync.dma_start(
                            outT[ct * 128:(ct + 1) * 128,
                                 c * SBLK:(c + 1) * SBLK],
                            ob[:])

                # Stagger o_proj two chunks behind attention so each chunk's
                # AllGather has ~2 attention blocks of slack to complete.
                attn_block(0)
                ag_block(0)
                attn_block(1)
                ag_block(1)
                attn_block(2)
                oproj_block(0)
                ag_block(2)
                attn_block(3)
                oproj_block(1)
                ag_block(3)
                oproj_block(2)
                oproj_block(3)


_NC_CACHE = None


def build_program():
    global _NC_CACHE
    if _NC_CACHE is not None:
        return _NC_CACHE
    nc = bacc.Bacc("TRN2", target_bir_lowering=False, debug=False,
                   enable_asserts=False, num_devices=NCORES)
    with tile.TileContext(nc) as tc:
        _emit(tc)
    nc.compile()
    _NC_CACHE = nc
    return nc


def _prep_inputs(x, cos, sin, Wq, Wk, Wv, Wo):
    """Build the 8 per-core input maps (host-side sharding only)."""
    x = np.asarray(x, dtype=np.float32)
    cos = np.asarray(cos, dtype=np.float32)
    sin = np.asarray(sin, dtype=np.float32)
    Wq = np.asarray(Wq, dtype=np.float32)
    Wk = np.asarray(Wk, dtype=np.float32)
    Wv = np.asarray(Wv, dtype=np.float32)
    Wo = np.asarray(Wo, dtype=np.float32)

    cosT = np.ascontiguousarray(cos.T).astype(NPDT)             # [128, S]
    sinT = np.ascontiguousarray(sin.T)
    srotT = np.concatenate([-sinT[:64], sinT[64:]], axis=0).astype(NPDT)

    iota = np.arange(SBLK)[None, :]
    rows = np.arange(128)[:, None]
    masks = np.stack(
        [(128 * p + rows <= iota) for p in range(4)]).astype(NPDT)  # [4,128,512]

    xTg = [np.ascontiguousarray(x[g].T).astype(NPDT) for g in range(B)]

    in_maps = []
    for c in range(NCORES):
        g, r = c // TP, c % TP
        hs = slice(r * HPC * DH, (r + 1) * HPC * DH)
        in_maps.append({
            "xT": xTg[g],
            "wqT": np.ascontiguousarray(Wq[hs].T).astype(NPDT),
            "wkT": np.ascontiguousarray(Wk[hs].T).astype(NPDT),
            "wvT": np.ascontiguousarray(Wv[hs].T).astype(NPDT),
            "woT": np.ascontiguousarray(Wo[r * OCOLS:(r + 1) * OCOLS].T).astype(NPDT),
            "cosT": cosT,
            "srotT": srotT,
            "masks": masks,
        })
    return in_maps


def run(inputs, trace=False, trace_cores=None):
    nc = build_program()
    in_maps = _prep_inputs(**inputs)
    res = run_bass_kernel_spmd(
        nc, in_maps, core_ids=list(range(NCORES)),
        trace=trace, trace_cores=trace_cores,
    )
    out = np.empty((B, S, D), dtype=np.float32)
    for c in range(NCORES):
        g, r = c // TP, c % TP
        out[g, :, r * OCOLS:(r + 1) * OCOLS] = res.results[c]["outT"].T
    return out, res


def kernel(**inputs):
    out, _ = run(inputs)
    return out


# revision 14
# speedup vs baseline: 1.3457x; 1.1565x over previous
"""Llama attention layer (B=2, S=2048, D=2048, H=16, DH=128) on 8 TRN2 NeuronCores.

Sharding: 2-way data parallel over batch x 4-way tensor parallel over heads.
Core c: batch g = c // 4, heads 4r..4r+3 where r = c % 4.
Projections are column-parallel (each core computes Q/K/V for its 4 heads),
attention is fully local per (batch, head), then the per-head attention
outputs (kept transposed, [dim, seq]) are AllGather'd within each 4-core
batch group in 4 seq-chunks (pipelined against attention compute), and
o_proj is column-parallel: core c computes output columns r*512..(r+1)*512
of its batch. Host concatenates - no host-side compute.

All matmul operands are bf16 (fp32 accumulation in PSUM); softmax runs
without max-subtraction (scores are O(6), exp is safe in fp32); the
denominator is accumulated on the vector engine in fp32 and reduced over
partitions with a single ones-matmul (f32r) per (head, seq-block).
"""

import os
import sys

for _p in ("/opt/trn_rl_repo", "/root/.axon_site/_ro/trn_rl_repo"):
    if os.path.isdir(_p) and _p not in sys.path:
        sys.path.append(_p)

import numpy as np
import ml_dtypes

import concourse.bass as bass
import concourse.tile as tile
import concourse.mybir as mybir
from concourse import bacc
from concourse.bass_utils import run_bass_kernel_spmd

F32 = mybir.dt.float32
F32R = mybir.dt.float32r
BF16 = mybir.dt.bfloat16
AF = mybir.ActivationFunctionType

B, S, D, H, DH = 2, 2048, 2048, 16, 128
NCORES = 8
TP = 4                 # cores per batch group
HPC = H // TP          # heads per core = 4
SBLK = 512             # seq block (matmul moving size)
NSB = S // SBLK        # 4
DTILES = D // 128      # 16 contraction tiles
KT = S // 128          # 16 key tiles
OCOLS = D // TP        # 512 output columns per core
SCALE = 1.0 / float(np.sqrt(DH))
DMA_SPLIT = 4          # split big input DMAs so compute starts early

DT = BF16              # matmul operand dtype
NPDT = ml_dtypes.bfloat16


def _split_load(nc, dst, src_2d, inner):
    """DMA a [D, inner] DRAM tensor into dst [128, DTILES, inner] in
    DMA_SPLIT chunks along the d-tile axis (lets consumers of early
    d-tiles start before the whole tensor has landed)."""
    step = DTILES // DMA_SPLIT
    for i in range(DMA_SPLIT):
        t0 = i * step
        nc.sync.dma_start(
            dst[:, t0:t0 + step, :],
            src_2d[t0 * 128:(t0 + step) * 128, :].rearrange(
                "(t p) s -> p t s", p=128),
        )


def _emit(tc):
    nc = tc.nc
    xT = nc.dram_tensor("xT", [D, S], DT, kind="ExternalInput").ap()
    wqT = nc.dram_tensor("wqT", [D, HPC * DH], DT, kind="ExternalInput").ap()
    wkT = nc.dram_tensor("wkT", [D, HPC * DH], DT, kind="ExternalInput").ap()
    wvT = nc.dram_tensor("wvT", [D, HPC * DH], DT, kind="ExternalInput").ap()
    woT = nc.dram_tensor("woT", [D, OCOLS], DT, kind="ExternalInput").ap()
    cosT = nc.dram_tensor("cosT", [DH, S], DT, kind="ExternalInput").ap()
    srotT = nc.dram_tensor("srotT", [DH, S], DT, kind="ExternalInput").ap()
    masks = nc.dram_tensor("masks", [4, 128, SBLK], DT, kind="ExternalInput").ap()
    outT = nc.dram_tensor("outT", [OCOLS, S], F32, kind="ExternalOutput").ap()

    # Per-seq-chunk bounce buffers for the pipelined AllGather. The last
    # attention block is split into two half-chunks to shrink the serial
    # AG + o_proj tail.
    chunks = [(0, 512), (512, 512), (1024, 512), (1536, 256), (1792, 256)]
    vloc = [nc.dram_tensor(f"vals_loc_{c}", [HPC * DH, w], DT).ap()
            for c, (_, w) in enumerate(chunks)]
    vgath = [nc.dram_tensor(f"vals_gath_{c}", [D, w], DT).ap()
             for c, (_, w) in enumerate(chunks)]

    with tc.tile_pool(name="const", bufs=1) as cpool:
        cos_s = cpool.tile([128, S], DT, name="cos_s")
        srot_s = cpool.tile([128, S], DT, name="srot_s")
        mask_s = cpool.tile([128, 4, SBLK], DT, name="mask_s")
        # bf16 ones matrix: ones.T @ x sums x over partitions and yields the
        # result replicated across all 128 partitions (DVE cannot broadcast
        # along partitions, so produce the softmax denominator pre-broadcast).
        ones_b = cpool.tile([128, 128], DT, name="ones_b")
        nc.vector.memset(ones_b[:], 1.0)

        with tc.tile_pool(name="qkv", bufs=1) as qkvpool:
            qT = qkvpool.tile([128, HPC, S], DT, name="qT")
            kTt = qkvpool.tile([128, HPC, S], DT, name="kTt")
            v_s = qkvpool.tile([128, KT, HPC * DH], DT, name="v_s")

            # ---------------- projections + RoPE ----------------
            with tc.tile_pool(name="wqkv", bufs=1) as wpool, \
                 tc.tile_pool(name="xs", bufs=2) as xpool, \
                 tc.tile_pool(name="pp", bufs=2, space="PSUM") as ppool, \
                 tc.tile_pool(name="rope", bufs=2) as rpool:
                wq_s = wpool.tile([128, DTILES, HPC * DH], DT, name="wq_s")
                wk_s = wpool.tile([128, DTILES, HPC * DH], DT, name="wk_s")
                wv_s = wpool.tile([128, DTILES, HPC * DH], DT, name="wv_s")
                # Load order tuned for startup latency: the first x chunk and
                # the first Wq/Wk chunks come first so the QK sweep can begin
                # within a few microseconds of kernel start.
                x0 = xpool.tile([128, DTILES, SBLK], DT, tag="x", name="x_s")
                step = DTILES // DMA_SPLIT

                def _chunk(dst, src_2d, i):
                    t0 = i * step
                    nc.sync.dma_start(
                        dst[:, t0:t0 + step, :],
                        src_2d[t0 * 128:(t0 + step) * 128, :].rearrange(
                            "(t p) s -> p t s", p=128))

                _chunk(x0, xT[:, 0:SBLK], 0)
                _chunk(wq_s, wqT, 0)
                _chunk(wk_s, wkT, 0)
                for i in range(1, DMA_SPLIT):
                    _chunk(x0, xT[:, 0:SBLK], i)
                    _chunk(wq_s, wqT, i)
                    _chunk(wk_s, wkT, i)
                _split_load(nc, wv_s, wvT, HPC * DH)
                nc.sync.dma_start(cos_s[:], cosT[:, :])
                nc.sync.dma_start(srot_s[:], srotT[:, :])
                nc.sync.dma_start(mask_s[:], masks.rearrange("m p s -> p m s"))

                for sb in range(NSB):
                    s0 = sb * SBLK
                    if sb == 0:
                        x_s = x0
                    else:
                        x_s = xpool.tile([128, DTILES, SBLK], DT, tag="x",
                                         name="x_s")
                        _split_load(nc, x_s, xT[:, s0:s0 + SBLK], SBLK)
                    # Q / K sweep: 8 PSUM accumulators (2 tiles x 4 banks)
                    psq = ppool.tile([128, HPC, SBLK], F32, tag="ps", name="psq")
                    psk = ppool.tile([128, HPC, SBLK], F32, tag="ps", name="psk")
                    for dt_i in range(DTILES):
                        st_ = dt_i == 0
                        sp_ = dt_i == DTILES - 1
                        for h in range(HPC):
                            nc.tensor.matmul(
                                psq[:, h, :],
                                lhsT=wq_s[:, dt_i, h * DH:(h + 1) * DH],
                                rhs=x_s[:, dt_i, :],
                                start=st_, stop=sp_,
                            )
                        for h in range(HPC):
                            nc.tensor.matmul(
                                psk[:, h, :],
                                lhsT=wk_s[:, dt_i, h * DH:(h + 1) * DH],
                                rhs=x_s[:, dt_i, :],
                                start=st_, stop=sp_,
                            )

                    # RoPE: out = raw*cos + rot(raw)*srot  (partition dim = dh)
                    cos_b = cos_s[:, s0:s0 + SBLK].unsqueeze(1).broadcast_to(
                        [128, HPC, SBLK])
                    srot_b = srot_s[:, s0:s0 + SBLK].unsqueeze(1).broadcast_to(
                        [128, HPC, SBLK])
                    for ps, dst in ((psq, qT), (psk, kTt)):
                        raw = rpool.tile([128, HPC, SBLK], DT, tag="raw", name="raw")
                        nc.scalar.copy(raw[:], ps[:])
                        # rotate-half along partitions: engines can't shift
                        # partitions, DMA can.
                        rot = rpool.tile([128, HPC, SBLK], DT, tag="rot", name="rot")
                        nc.scalar.dma_start(rot[0:64], raw[64:128])
                        nc.scalar.dma_start(rot[64:128], raw[0:64])
                        rs = rpool.tile([128, HPC, SBLK], DT, tag="rs", name="rs")
                        nc.vector.tensor_mul(rs[:], rot[:], srot_b)
                        qc = rpool.tile([128, HPC, SBLK], DT, tag="qc", name="qc")
                        nc.vector.tensor_mul(qc[:], raw[:], cos_b)
                        nc.vector.tensor_add(dst[:, :, s0:s0 + SBLK], qc[:], rs[:])

                    # V sweep: x^T tiles stationary, W_v moving
                    psv = ppool.tile([128, 4, SBLK], F32, tag="ps", name="psv")
                    for dt_i in range(DTILES):
                        st_ = dt_i == 0
                        sp_ = dt_i == DTILES - 1
                        for st in range(4):  # seq sub-tiles of this block
                            nc.tensor.matmul(
                                psv[:, st, :],
                                lhsT=x_s[:, dt_i, st * 128:(st + 1) * 128],
                                rhs=wv_s[:, dt_i, :],
                                start=st_, stop=sp_,
                            )
                    for st in range(4):
                        nc.scalar.copy(v_s[:, sb * 4 + st, :], psv[:, st, :])

            # ---------------- attention + AG + o_proj, chunk-pipelined ------
            with tc.tile_pool(name="aps", bufs=2, space="PSUM") as apsum, \
                 tc.tile_pool(name="att", bufs=3) as apool, \
                 tc.tile_pool(name="og", bufs=1) as ogpool, \
                 tc.tile_pool(name="ops", bufs=2, space="PSUM") as opsum, \
                 tc.tile_pool(name="ob", bufs=3) as obpool:

                wo_s = ogpool.tile([128, DTILES, OCOLS], DT, name="wo_s")
                _split_load(nc, wo_s, woT, OCOLS)

                def attn_block(sqb):
                    sq0 = sqb * SBLK
                    nkt = 4 * (sqb + 1)
                    for h in range(HPC):
                        ps_av = apsum.tile([128, SBLK], F32, tag="av",
                                           name="ps_av")
                        ps_den = apsum.tile([128, SBLK], F32, tag="den",
                                            name="ps_den", bufs=1)
                        for q in range(nkt // 4):
                            # quad-sum of exp tiles (bf16): 3 DVE adds per 4
                            # k-tiles; one ones-matmul per quad accumulates
                            # the softmax denominator in PSUM (partition-
                            # replicated).
                            qsum = apool.tile([128, SBLK], DT, tag="qsum",
                                              name="qsum", bufs=2)
                            for j in range(2):
                                ps_st = apsum.tile([128, 2, SBLK], F32,
                                                   tag="st", name="ps_st",
                                                   bufs=2)
                                for i in range(2):
                                    kt = 4 * q + 2 * j + i
                                    nc.tensor.matmul(
                                        ps_st[:, i, :],
                                        lhsT=kTt[:, h, kt * 128:(kt + 1) * 128],
                                        rhs=qT[:, h, sq0:sq0 + SBLK],
                                        start=True, stop=True,
                                    )
                                st_e = apool.tile([128, 2, SBLK], DT,
                                                  tag="ste", name="st_e",
                                                  bufs=3)
                                nc.scalar.activation(st_e[:], ps_st[:], AF.Exp,
                                                     scale=SCALE)
                                p = 4 * q + 2 * j - (nkt - 4)
                                if p >= 0:  # diagonal pair: causal 0/1 mask
                                    nc.vector.tensor_mul(
                                        st_e[:], st_e[:],
                                        mask_s[:, p:p + 2, :])
                                for i in range(2):
                                    kt = 4 * q + 2 * j + i
                                    nc.tensor.matmul(
                                        ps_av[:],
                                        lhsT=v_s[:, kt, h * DH:(h + 1) * DH],
                                        rhs=st_e[:, i, :],
                                        start=(kt == 0), stop=(kt == nkt - 1),
                                    )
                                if j == 0:
                                    nc.vector.tensor_add(qsum[:], st_e[:, 0, :],
                                                         st_e[:, 1, :])
                                else:
                                    nc.vector.tensor_add(qsum[:], qsum[:],
                                                         st_e[:, 0, :])
                                    nc.vector.tensor_add(qsum[:], qsum[:],
                                                         st_e[:, 1, :])
                            nc.tensor.matmul(
                                ps_den[:],
                                lhsT=ones_b[:],
                                rhs=qsum[:],
                                start=(q == 0), stop=(q == nkt // 4 - 1),
                            )
                        rden = apool.tile([128, SBLK], F32, tag="rden",
                                          name="rden", bufs=2)
                        nc.vector.reciprocal_approx_fast(rden[:], ps_den[:])
                        vout = apool.tile([128, SBLK], DT, tag="vout",
                                          name="vout")
                        nc.vector.tensor_mul(vout[:], ps_av[:], rden[:])
                        # scatter into the AG chunk buffers covering this block
                        for c, (off, w) in enumerate(chunks):
                            lo, hi = max(off, sq0), min(off + w, sq0 + SBLK)
                            if lo < hi:
                                nc.sync.dma_start(
                                    vloc[c][h * DH:(h + 1) * DH, lo - off:hi - off],
                                    vout[:, lo - sq0:hi - sq0])

                def ag_block(c):
                    nc.gpsimd.collective_compute(
                        "AllGather",
                        mybir.AluOpType.bypass,
                        replica_groups=[[0, 1, 2, 3], [4, 5, 6, 7]],
                        ins=[vloc[c][:, :].opt()],
                        outs=[vgath[c][:, :].opt()],
                    )

                def oproj_block(c):
                    off, w = chunks[c]
                    vg = ogpool.tile([128, DTILES, SBLK], DT, tag="vg",
                                     name="vg", bufs=2)
                    _split_load(nc, vg[:, :, 0:w], vgath[c], w)
                    for ct in range(OCOLS // 128):
                        ps_o = opsum.tile([128, SBLK], F32, tag="o", name="ps_o",
                                          bufs=1)
                        for dt_i in range(DTILES):
                            nc.tensor.matmul(
                                ps_o[:, 0:w],
                                lhsT=wo_s[:, dt_i, ct * 128:(ct + 1) * 128],
                                rhs=vg[:, dt_i, 0:w],
                                start=(dt_i == 0), stop=(dt_i == DTILES - 1),
                            )
                        ob = obpool.tile([128, SBLK], F32, tag="ob", name="ob")
                        nc.scalar.copy(ob[:, 0:w], ps_o[:, 0:w])
                        nc.scalar.dma_start(
                            outT[ct * 128:(ct + 1) * 128, off:off + w],
                            ob[:, 0:w])

                # Stagger o_proj behind attention so each chunk's
                # AllGather has attention-compute slack to complete under.
                attn_block(0)
                ag_block(0)
                attn_block(1)
                ag_block(1)
                attn_block(2)
                oproj_block(0)
                ag_block(2)
                attn_block(3)
                oproj_block(1)
                ag_block(3)
                ag_block(4)
                oproj_block(2)
                oproj_block(3)
                oproj_block(4)


_NC_CACHE = None


def build_program():
    global _NC_CACHE
    if _NC_CACHE is not None:
        return _NC_CACHE
    nc = bacc.Bacc("TRN2", target_bir_lowering=False, debug=False,
                   enable_asserts=False, num_devices=NCORES)
    with tile.TileContext(nc) as tc:
        _emit(tc)
    nc.compile()
    _NC_CACHE = nc
    return nc


def _prep_inputs(x, cos, sin, Wq, Wk, Wv, Wo):
    """Build the 8 per-core input maps (host-side sharding only)."""
    x = np.asarray(x, dtype=np.float32)
    cos = np.asarray(cos, dtype=np.float32)
    sin = np.asarray(sin, dtype=np.float32)
    Wq = np.asarray(Wq, dtype=np.float32)
    Wk = np.asarray(Wk, dtype=np.float32)
    Wv = np.asarray(Wv, dtype=np.float32)
    Wo = np.asarray(Wo, dtype=np.float32)

    cosT = np.ascontiguousarray(cos.T).astype(NPDT)             # [128, S]
    sinT = np.ascontiguousarray(sin.T)
    srotT = np.concatenate([-sinT[:64], sinT[64:]], axis=0).astype(NPDT)

    iota = np.arange(SBLK)[None, :]
    rows = np.arange(128)[:, None]
    masks = np.stack(
        [(128 * p + rows <= iota) for p in range(4)]).astype(NPDT)  # [4,128,512]

    xTg = [np.ascontiguousarray(x[g].T).astype(NPDT) for g in range(B)]

    in_maps = []
    for c in range(NCORES):
        g, r = c // TP, c % TP
        hs = slice(r * HPC * DH, (r + 1) * HPC * DH)
        in_maps.append({
            "xT": xTg[g],
            "wqT": np.ascontiguousarray(Wq[hs].T).astype(NPDT),
            "wkT": np.ascontiguousarray(Wk[hs].T).astype(NPDT),
            "wvT": np.ascontiguousarray(Wv[hs].T).astype(NPDT),
            "woT": np.ascontiguousarray(Wo[r * OCOLS:(r + 1) * OCOLS].T).astype(NPDT),
            "cosT": cosT,
            "srotT": srotT,
            "masks": masks,
        })
    return in_maps


def run(inputs, trace=False, trace_cores=None):
    nc = build_program()
    in_maps = _prep_inputs(**inputs)
    res = run_bass_kernel_spmd(
        nc, in_maps, core_ids=list(range(NCORES)),
        trace=trace, trace_cores=trace_cores,
    )
    out = np.empty((B, S, D), dtype=np.float32)
    for c in range(NCORES):
        g, r = c // TP, c % TP
        out[g, :, r * OCOLS:(r + 1) * OCOLS] = res.results[c]["outT"].T
    return out, res


def kernel(**inputs):
    out, _ = run(inputs)
    return out


# revision 15
# speedup vs baseline: 1.3962x; 1.0375x over previous
"""Llama attention layer (B=2, S=2048, D=2048, H=16, DH=128) on 8 TRN2 NeuronCores.

Sharding: 2-way data parallel over batch x 4-way tensor parallel over heads.
Core c: batch g = c // 4, heads 4r..4r+3 where r = c % 4.
Projections are column-parallel (each core computes Q/K/V for its 4 heads),
attention is fully local per (batch, head), then the per-head attention
outputs (kept transposed, [dim, seq]) are AllGather'd within each 4-core
batch group in 4 seq-chunks (pipelined against attention compute), and
o_proj is column-parallel: core c computes output columns r*512..(r+1)*512
of its batch. Host concatenates - no host-side compute.

All matmul operands are bf16 (fp32 accumulation in PSUM); softmax runs
without max-subtraction (scores are O(6), exp is safe in fp32); the
denominator is accumulated on the vector engine in fp32 and reduced over
partitions with a single ones-matmul (f32r) per (head, seq-block).
"""

import os
import sys

for _p in ("/opt/trn_rl_repo", "/root/.axon_site/_ro/trn_rl_repo"):
    if os.path.isdir(_p) and _p not in sys.path:
        sys.path.append(_p)

import numpy as np
import ml_dtypes

import concourse.bass as bass
import concourse.tile as tile
import concourse.mybir as mybir
from concourse import bacc
from concourse.bass_utils import run_bass_kernel_spmd

F32 = mybir.dt.float32
F32R = mybir.dt.float32r
BF16 = mybir.dt.bfloat16
AF = mybir.ActivationFunctionType

B, S, D, H, DH = 2, 2048, 2048, 16, 128
NCORES = 8
TP = 4                 # cores per batch group
HPC = H // TP          # heads per core = 4
SBLK = 512             # seq block (matmul moving size)
NSB = S // SBLK        # 4
DTILES = D // 128      # 16 contraction tiles
KT = S // 128          # 16 key tiles
OCOLS = D // TP        # 512 output columns per core
SCALE = 1.0 / float(np.sqrt(DH))
DMA_SPLIT = 4          # split big input DMAs so compute starts early

DT = BF16              # matmul operand dtype
NPDT = ml_dtypes.bfloat16


def _split_load(nc, dst, src_2d, inner):
    """DMA a [D, inner] DRAM tensor into dst [128, DTILES, inner] in
    DMA_SPLIT chunks along the d-tile axis (lets consumers of early
    d-tiles start before the whole tensor has landed)."""
    step = DTILES // DMA_SPLIT
    for i in range(DMA_SPLIT):
        t0 = i * step
        nc.sync.dma_start(
            dst[:, t0:t0 + step, :],
            src_2d[t0 * 128:(t0 + step) * 128, :].rearrange(
                "(t p) s -> p t s", p=128),
        )


def _emit(tc):
    nc = tc.nc
    xT = nc.dram_tensor("xT", [D, S], DT, kind="ExternalInput").ap()
    wqT = nc.dram_tensor("wqT", [D, HPC * DH], DT, kind="ExternalInput").ap()
    wkT = nc.dram_tensor("wkT", [D, HPC * DH], DT, kind="ExternalInput").ap()
    wvT = nc.dram_tensor("wvT", [D, HPC * DH], DT, kind="ExternalInput").ap()
    woT = nc.dram_tensor("woT", [D, OCOLS], DT, kind="ExternalInput").ap()
    cosT = nc.dram_tensor("cosT", [DH, S], DT, kind="ExternalInput").ap()
    srotT = nc.dram_tensor("srotT", [DH, S], DT, kind="ExternalInput").ap()
    masks = nc.dram_tensor("masks", [4, 128, SBLK], DT, kind="ExternalInput").ap()
    outT = nc.dram_tensor("outT", [OCOLS, S], F32, kind="ExternalOutput").ap()

    # Per-seq-chunk bounce buffers for the pipelined AllGather. The last
    # attention block is split into two half-chunks to shrink the serial
    # AG + o_proj tail.
    chunks = [(0, 512), (512, 512), (1024, 512), (1536, 256), (1792, 256)]
    vloc = [nc.dram_tensor(f"vals_loc_{c}", [HPC * DH, w], DT).ap()
            for c, (_, w) in enumerate(chunks)]
    vgath = [nc.dram_tensor(f"vals_gath_{c}", [D, w], DT).ap()
             for c, (_, w) in enumerate(chunks)]
    wup_in = nc.dram_tensor("wup_in", [128, 4], DT).ap()
    wup_out = nc.dram_tensor("wup_out", [512, 4], DT).ap()

    with tc.tile_pool(name="const", bufs=1) as cpool:
        cos_s = cpool.tile([128, S], DT, name="cos_s")
        srot_s = cpool.tile([128, S], DT, name="srot_s")
        mask_s = cpool.tile([128, 4, SBLK], DT, name="mask_s")
        # bf16 ones matrix: ones.T @ x sums x over partitions and yields the
        # result replicated across all 128 partitions (DVE cannot broadcast
        # along partitions, so produce the softmax denominator pre-broadcast).
        ones_b = cpool.tile([128, 128], DT, name="ones_b")
        nc.vector.memset(ones_b[:], 1.0)
        # Tiny warm-up AllGather: the first collective of an execution pays
        # ~40us of one-time overhead; absorb it during the projection phase.
        nc.gpsimd.dma_start(wup_in[:, :], ones_b[:, 0:4])
        nc.gpsimd.collective_compute(
            "AllGather", mybir.AluOpType.bypass,
            replica_groups=[[0, 1, 2, 3], [4, 5, 6, 7]],
            ins=[wup_in[:, :].opt()], outs=[wup_out[:, :].opt()],
        )

        with tc.tile_pool(name="qkv", bufs=1) as qkvpool:
            qT = qkvpool.tile([128, HPC, S], DT, name="qT")
            kTt = qkvpool.tile([128, HPC, S], DT, name="kTt")
            v_s = qkvpool.tile([128, KT, HPC * DH], DT, name="v_s")

            # ---------------- projections + RoPE ----------------
            with tc.tile_pool(name="wqkv", bufs=1) as wpool, \
                 tc.tile_pool(name="xs", bufs=2) as xpool, \
                 tc.tile_pool(name="pp", bufs=2, space="PSUM") as ppool, \
                 tc.tile_pool(name="rope", bufs=2) as rpool:
                wq_s = wpool.tile([128, DTILES, HPC * DH], DT, name="wq_s")
                wk_s = wpool.tile([128, DTILES, HPC * DH], DT, name="wk_s")
                wv_s = wpool.tile([128, DTILES, HPC * DH], DT, name="wv_s")
                # Load order tuned for startup latency: the first x chunk and
                # the first Wq/Wk chunks come first so the QK sweep can begin
                # within a few microseconds of kernel start.
                x0 = xpool.tile([128, DTILES, SBLK], DT, tag="x", name="x_s")
                step = DTILES // DMA_SPLIT

                def _chunk(dst, src_2d, i):
                    t0 = i * step
                    nc.sync.dma_start(
                        dst[:, t0:t0 + step, :],
                        src_2d[t0 * 128:(t0 + step) * 128, :].rearrange(
                            "(t p) s -> p t s", p=128))

                _chunk(x0, xT[:, 0:SBLK], 0)
                _chunk(wq_s, wqT, 0)
                _chunk(wk_s, wkT, 0)
                for i in range(1, DMA_SPLIT):
                    _chunk(x0, xT[:, 0:SBLK], i)
                    _chunk(wq_s, wqT, i)
                    _chunk(wk_s, wkT, i)
                _split_load(nc, wv_s, wvT, HPC * DH)
                nc.sync.dma_start(cos_s[:], cosT[:, :])
                nc.sync.dma_start(srot_s[:], srotT[:, :])
                nc.sync.dma_start(mask_s[:], masks.rearrange("m p s -> p m s"))

                for sb in range(NSB):
                    s0 = sb * SBLK
                    if sb == 0:
                        x_s = x0
                    else:
                        x_s = xpool.tile([128, DTILES, SBLK], DT, tag="x",
                                         name="x_s")
                        _split_load(nc, x_s, xT[:, s0:s0 + SBLK], SBLK)
                    # Q / K sweep: 8 PSUM accumulators (2 tiles x 4 banks)
                    psq = ppool.tile([128, HPC, SBLK], F32, tag="ps", name="psq")
                    psk = ppool.tile([128, HPC, SBLK], F32, tag="ps", name="psk")
                    for dt_i in range(DTILES):
                        st_ = dt_i == 0
                        sp_ = dt_i == DTILES - 1
                        for h in range(HPC):
                            nc.tensor.matmul(
                                psq[:, h, :],
                                lhsT=wq_s[:, dt_i, h * DH:(h + 1) * DH],
                                rhs=x_s[:, dt_i, :],
                                start=st_, stop=sp_,
                            )
                        for h in range(HPC):
                            nc.tensor.matmul(
                                psk[:, h, :],
                                lhsT=wk_s[:, dt_i, h * DH:(h + 1) * DH],
                                rhs=x_s[:, dt_i, :],
                                start=st_, stop=sp_,
                            )

                    # RoPE: out = raw*cos + rot(raw)*srot  (partition dim = dh)
                    cos_b = cos_s[:, s0:s0 + SBLK].unsqueeze(1).broadcast_to(
                        [128, HPC, SBLK])
                    srot_b = srot_s[:, s0:s0 + SBLK].unsqueeze(1).broadcast_to(
                        [128, HPC, SBLK])
                    for ps, dst in ((psq, qT), (psk, kTt)):
                        raw = rpool.tile([128, HPC, SBLK], DT, tag="raw", name="raw")
                        nc.scalar.copy(raw[:], ps[:])
                        # rotate-half along partitions: engines can't shift
                        # partitions, DMA can.
                        rot = rpool.tile([128, HPC, SBLK], DT, tag="rot", name="rot")
                        nc.scalar.dma_start(rot[0:64], raw[64:128])
                        nc.scalar.dma_start(rot[64:128], raw[0:64])
                        rs = rpool.tile([128, HPC, SBLK], DT, tag="rs", name="rs")
                        nc.vector.tensor_mul(rs[:], rot[:], srot_b)
                        qc = rpool.tile([128, HPC, SBLK], DT, tag="qc", name="qc")
                        nc.vector.tensor_mul(qc[:], raw[:], cos_b)
                        nc.vector.tensor_add(dst[:, :, s0:s0 + SBLK], qc[:], rs[:])

                    # V sweep: x^T tiles stationary, W_v moving
                    psv = ppool.tile([128, 4, SBLK], F32, tag="ps", name="psv")
                    for dt_i in range(DTILES):
                        st_ = dt_i == 0
                        sp_ = dt_i == DTILES - 1
                        for st in range(4):  # seq sub-tiles of this block
                            nc.tensor.matmul(
                                psv[:, st, :],
                                lhsT=x_s[:, dt_i, st * 128:(st + 1) * 128],
                                rhs=wv_s[:, dt_i, :],
                                start=st_, stop=sp_,
                            )
                    for st in range(4):
                        nc.scalar.copy(v_s[:, sb * 4 + st, :], psv[:, st, :])

            # ---------------- attention + AG + o_proj, chunk-pipelined ------
            with tc.tile_pool(name="aps", bufs=2, space="PSUM") as apsum, \
                 tc.tile_pool(name="att", bufs=3) as apool, \
                 tc.tile_pool(name="og", bufs=1) as ogpool, \
                 tc.tile_pool(name="ops", bufs=2, space="PSUM") as opsum, \
                 tc.tile_pool(name="ob", bufs=3) as obpool:

                wo_s = ogpool.tile([128, DTILES, OCOLS], DT, name="wo_s")
                _split_load(nc, wo_s, woT, OCOLS)

                def attn_block(sqb):
                    sq0 = sqb * SBLK
                    nkt = 4 * (sqb + 1)
                    for h in range(HPC):
                        ps_av = apsum.tile([128, SBLK], F32, tag="av",
                                           name="ps_av")
                        ps_den = apsum.tile([128, SBLK], F32, tag="den",
                                            name="ps_den", bufs=1)
                        for q in range(nkt // 4):
                            # quad-sum of exp tiles (bf16): 3 DVE adds per 4
                            # k-tiles; one ones-matmul per quad accumulates
                            # the softmax denominator in PSUM (partition-
                            # replicated).
                            qsum = apool.tile([128, SBLK], DT, tag="qsum",
                                              name="qsum", bufs=2)
                            for j in range(2):
                                ps_st = apsum.tile([128, 2, SBLK], F32,
                                                   tag="st", name="ps_st",
                                                   bufs=2)
                                for i in range(2):
                                    kt = 4 * q + 2 * j + i
                                    nc.tensor.matmul(
                                        ps_st[:, i, :],
                                        lhsT=kTt[:, h, kt * 128:(kt + 1) * 128],
                                        rhs=qT[:, h, sq0:sq0 + SBLK],
                                        start=True, stop=True,
                                    )
                                st_e = apool.tile([128, 2, SBLK], DT,
                                                  tag="ste", name="st_e",
                                                  bufs=3)
                                nc.scalar.activation(st_e[:], ps_st[:], AF.Exp,
                                                     scale=SCALE)
                                p = 4 * q + 2 * j - (nkt - 4)
                                if p >= 0:  # diagonal pair: causal 0/1 mask
                                    nc.vector.tensor_mul(
                                        st_e[:], st_e[:],
                                        mask_s[:, p:p + 2, :])
                                for i in range(2):
                                    kt = 4 * q + 2 * j + i
                                    nc.tensor.matmul(
                                        ps_av[:],
                                        lhsT=v_s[:, kt, h * DH:(h + 1) * DH],
                                        rhs=st_e[:, i, :],
                                        start=(kt == 0), stop=(kt == nkt - 1),
                                    )
                                if j == 0:
                                    nc.vector.tensor_add(qsum[:], st_e[:, 0, :],
                                                         st_e[:, 1, :])
                                else:
                                    nc.vector.tensor_add(qsum[:], qsum[:],
                                                         st_e[:, 0, :])
                                    nc.vector.tensor_add(qsum[:], qsum[:],
                                                         st_e[:, 1, :])
                            nc.tensor.matmul(
                                ps_den[:],
                                lhsT=ones_b[:],
                                rhs=qsum[:],
                                start=(q == 0), stop=(q == nkt // 4 - 1),
                            )
                        rden = apool.tile([128, SBLK], F32, tag="rden",
                                          name="rden", bufs=2)
                        nc.vector.reciprocal_approx_fast(rden[:], ps_den[:])
                        vout = apool.tile([128, SBLK], DT, tag="vout",
                                          name="vout")
                        nc.vector.tensor_mul(vout[:], ps_av[:], rden[:])
                        # scatter into the AG chunk buffers covering this block
                        for c, (off, w) in enumerate(chunks):
                            lo, hi = max(off, sq0), min(off + w, sq0 + SBLK)
                            if lo < hi:
                                nc.sync.dma_start(
                                    vloc[c][h * DH:(h + 1) * DH, lo - off:hi - off],
                                    vout[:, lo - sq0:hi - sq0])

                def ag_block(c):
                    nc.gpsimd.collective_compute(
                        "AllGather",
                        mybir.AluOpType.bypass,
                        replica_groups=[[0, 1, 2, 3], [4, 5, 6, 7]],
                        ins=[vloc[c][:, :].opt()],
                        outs=[vgath[c][:, :].opt()],
                    )

                def oproj_block(c):
                    off, w = chunks[c]
                    vg = ogpool.tile([128, DTILES, SBLK], DT, tag="vg",
                                     name="vg", bufs=2)
                    # gpsimd queue: a vg load waits on its AllGather, and on
                    # the sync queue that wait would head-of-line-block the
                    # attention-output DMAs queued behind it.
                    for i_ in range(2):
                        t0 = i_ * (DTILES // 2)
                        nc.gpsimd.dma_start(
                            vg[:, t0:t0 + DTILES // 2, 0:w],
                            vgath[c][t0 * 128:(t0 + DTILES // 2) * 128, :]
                            .rearrange("(t p) s -> p t s", p=128))
                    for ct in range(OCOLS // 128):
                        ps_o = opsum.tile([128, SBLK], F32, tag="o", name="ps_o",
                                          bufs=1)
                        for dt_i in range(DTILES):
                            nc.tensor.matmul(
                                ps_o[:, 0:w],
                                lhsT=wo_s[:, dt_i, ct * 128:(ct + 1) * 128],
                                rhs=vg[:, dt_i, 0:w],
                                start=(dt_i == 0), stop=(dt_i == DTILES - 1),
                            )
                        ob = obpool.tile([128, SBLK], F32, tag="ob", name="ob")
                        nc.scalar.copy(ob[:, 0:w], ps_o[:, 0:w])
                        nc.scalar.dma_start(
                            outT[ct * 128:(ct + 1) * 128, off:off + w],
                            ob[:, 0:w])

                # Stagger o_proj behind attention so each chunk's
                # AllGather has attention-compute slack to complete under.
                attn_block(0)
                ag_block(0)
                attn_block(1)
                ag_block(1)
                attn_block(2)
                oproj_block(0)
                ag_block(2)
                attn_block(3)
                oproj_block(1)
                ag_block(3)
                ag_block(4)
                oproj_block(2)
                oproj_block(3)
                oproj_block(4)


_NC_CACHE = None


def build_program():
    global _NC_CACHE
    if _NC_CACHE is not None:
        return _NC_CACHE
    nc = bacc.Bacc("TRN2", target_bir_lowering=False, debug=False,
                   enable_asserts=False, num_devices=NCORES)
    with tile.TileContext(nc) as tc:
        _emit(tc)
    nc.compile()
    _NC_CACHE = nc
    return nc


def _prep_inputs(x, cos, sin, Wq, Wk, Wv, Wo):
    """Build the 8 per-core input maps (host-side sharding only)."""
    x = np.asarray(x, dtype=np.float32)
    cos = np.asarray(cos, dtype=np.float32)
    sin = np.asarray(sin, dtype=np.float32)
    Wq = np.asarray(Wq, dtype=np.float32)
    Wk = np.asarray(Wk, dtype=np.float32)
    Wv = np.asarray(Wv, dtype=np.float32)
    Wo = np.asarray(Wo, dtype=np.float32)

    cosT = np.ascontiguousarray(cos.T).astype(NPDT)             # [128, S]
    sinT = np.ascontiguousarray(sin.T)
    srotT = np.concatenate([-sinT[:64], sinT[64:]], axis=0).astype(NPDT)

    iota = np.arange(SBLK)[None, :]
    rows = np.arange(128)[:, None]
    masks = np.stack(
        [(128 * p + rows <= iota) for p in range(4)]).astype(NPDT)  # [4,128,512]

    xTg = [np.ascontiguousarray(x[g].T).astype(NPDT) for g in range(B)]

    in_maps = []
    for c in range(NCORES):
        g, r = c // TP, c % TP
        hs = slice(r * HPC * DH, (r + 1) * HPC * DH)
        in_maps.append({
            "xT": xTg[g],
            "wqT": np.ascontiguousarray(Wq[hs].T).astype(NPDT),
            "wkT": np.ascontiguousarray(Wk[hs].T).astype(NPDT),
            "wvT": np.ascontiguousarray(Wv[hs].T).astype(NPDT),
            "woT": np.ascontiguousarray(Wo[r * OCOLS:(r + 1) * OCOLS].T).astype(NPDT),
            "cosT": cosT,
            "srotT": srotT,
            "masks": masks,
        })
    return in_maps


def run(inputs, trace=False, trace_cores=None):
    nc = build_program()
    in_maps = _prep_inputs(**inputs)
    res = run_bass_kernel_spmd(
        nc, in_maps, core_ids=list(range(NCORES)),
        trace=trace, trace_cores=trace_cores,
    )
    out = np.empty((B, S, D), dtype=np.float32)
    for c in range(NCORES):
        g, r = c // TP, c % TP
        out[g, :, r * OCOLS:(r + 1) * OCOLS] = res.results[c]["outT"].T
    return out, res


def kernel(**inputs):
    out, _ = run(inputs)
    return out
